# revision 12
# baseline (speedup 1.0000x reference)
"""Trainium2 Bass kernel for nn_BasicBlock (binarized ResNet basic block).

Computation (see problem reference):
    residual = x
    out = psum_conv3x3(sign(x), sign(w1))        # 3x3 'same' conv, saturating acc
    out = bn1(out); out = hardtanh(out)
    out = psum_conv3x3(sign(out), sign(w2))
    out = bn2(out); out = out + residual; out = hardtanh(out)

Key facts exploited:
  * C=128 channels = one GROUP, 9 taps of |partial| <= 128 each, so the
    running accumulator magnitude is <= 9*128 = 1152 < THRESH=8000: the
    saturation clip NEVER binds. The conv is a plain 3x3 conv over sign
    values, exact in fp8 inputs with fp32 PSUM accumulation.
  * sign(hardtanh(v)) == sign(v), so the first hardtanh folds into the
    sign feeding conv2.
  * Each conv = 9 shifted-window taps (K=C=128 on partitions) into one PSUM
    accumulation group over a zero-padded row-stride-64 fp8 sign image.
    All 9 taps run as fp8 DoubleRow matmuls (5 per 8-row chunk): vertical
    neighbours (r0,c)+(r1,c) pair at plane stride RW; (r2,c0)+(r2,c1) pair
    via a byte-shifted copy of the image at +SHIFT; (r2,c2) pairs with a
    zero-weight dummy plane (a dummy DR is faster than a plain fp8 matmul).
  * Chunks are processed in fused pairs sharing a 2-bank PSUM tile, so each
    eviction (bn1+sign ACT, affine+clip DVE, shift-copy DMA) covers 16 rows
    in one instruction: fewer instructions -> fewer semaphores -> shorter
    runtime pre/postamble and less engine-issue overhead.
  * Host precomputes: the conv1 sign image (padded fp8; DMA'd twice at
    byte offsets 0/+1 to materialize both DR alignments with zero engine
    work), the bias-folded residual xb2 = x + (beta2 - mean2*inv2) in
    bf16, and the fp8 weight tables. Output is written bf16 and upcast
    on host (halves the write traffic; rel-err budget 2e-2 >> bf16 eps).

Sharding: data-parallel over batch: 64 images -> 8 cores x 8 images.
"""

import sys

sys.path.insert(0, "/opt/trn_rl_repo")

import numpy as np
import ml_dtypes

import concourse.bass as bass
import concourse.bacc as bacc
import concourse.mybir as mybir
import concourse.tile as tile
from concourse.bass_utils import run_bass_kernel_spmd

# ---------------------------------------------------------------- constants

N_CORES = 8
B, C, H, W = 64, 128, 56, 56
BL = B // N_CORES            # images per core
HP = H + 2                   # padded rows
RW = 64                      # padded row width; 64 keeps DR plane steps 16-aligned
CHUNK_ROWS = 8               # output rows per PSUM bank
NFLAT = CHUNK_ROWS * RW      # 512 flat psum columns per bank
EPS = 1e-5
SHIFT = HP * RW              # offset of the col-shifted copy inside xs/ts
T1B = SHIFT + 16             # host sign-image bytes per partition (pad tail)
WCOLS = 2 * 5 * 256          # fp8 weight table: 2 convs x 5 DoubleRow pairs
GROUPS = ((0, 1), (2, 3), (4, 5), (6,))

F32 = mybir.dt.float32
BF16 = mybir.dt.bfloat16
FP8 = mybir.dt.float8e4

_NC_CACHE = None


def _build_nc():
    """Build the per-core Bass module (same NEFF on all 8 cores)."""
    nc = bacc.Bacc("TRN2", debug=False)

    # host-prepped padded fp8 sign image of x, per image [C, HP*RW (+pad)]
    t1_d = nc.dram_tensor("t1", [BL, C, T1B], FP8, kind="ExternalInput").ap()
    # bias-folded bf16 residual x + b2
    xb2_d = nc.dram_tensor("xb2", [BL, C, H, W], BF16, kind="ExternalInput").ap()
    # fp8 weight tables, per conv: 5 DoubleRow pair tables [cin, 2*cout]
    w_d = nc.dram_tensor("w", [C, WCOLS], FP8, kind="ExternalInput").ap()
    # folded BN params per channel: [:,0]=inv1 [:,1]=b1 [:,2]=inv2
    bn_d = nc.dram_tensor("bn", [C, 4], F32, kind="ExternalInput").ap()
    y_d = nc.dram_tensor("y", [BL, C, H, W], BF16, kind="ExternalOutput").ap()

    SIGN = mybir.ActivationFunctionType.Sign
    DR = mybir.MatmulPerfMode.DoubleRow

    with tile.TileContext(nc) as tc:
        with (
            tc.tile_pool(name="const", bufs=1) as cpool,
            tc.tile_pool(name="xsign", bufs=3) as spool,
            tc.tile_pool(name="resid", bufs=2) as rpool,
            tc.tile_pool(name="outs", bufs=2) as opool,
            tc.tile_pool(name="psum", bufs=2, space="PSUM") as pspool,
        ):
            # weights first (gate the first matmul), then image-0's sign
            # image in pieces so the first conv group starts after the
            # first ~18 rows arrive instead of after the full transfer
            w_sb = cpool.tile([C, WCOLS], FP8)
            nc.sync.dma_start(w_sb[:], w_d[:])
            bn_sb = cpool.tile([C, 4], F32)
            nc.sync.dma_start(bn_sb[:], bn_d[:])
            CUT = 18 * RW
            xs0 = spool.tile([C, 2 * SHIFT], FP8, name="xs0")
            nc.sync.dma_start(xs0[:, 0:CUT], t1_d[0, :, 0:CUT])
            nc.sync.dma_start(
                xs0[:, SHIFT : SHIFT + CUT], t1_d[0, :, 1 : CUT + 1]
            )
            nc.sync.dma_start(xs0[:, CUT:SHIFT], t1_d[0, :, CUT:SHIFT])
            nc.sync.dma_start(
                xs0[:, SHIFT + CUT : 2 * SHIFT],
                t1_d[0, :, CUT + 1 : SHIFT + 1],
            )

            # conv2 input images: persistent double buffer; the main-copy
            # pads are zeroed once here and never overwritten afterwards
            # (bn1+sign writes rows 1..56 x cols 1..56, the shift DMA writes
            # only the +SHIFT copy).
            ts_bufs = []
            for t_i in range(2):
                ts = cpool.tile([C, 2 * SHIFT], FP8, name=f"ts{t_i}")
                ts3 = ts[:, 0:SHIFT].rearrange("p (h w) -> p h w", w=RW)
                nc.gpsimd.memset(ts3[:, 0, 0 : W + 2], 0.0)
                nc.gpsimd.memset(ts3[:, HP - 1, 0 : W + 2], 0.0)
                nc.gpsimd.memset(ts3[:, 1 : HP - 1, 0:1], 0.0)
                nc.gpsimd.memset(ts3[:, 1 : HP - 1, W + 1 : W + 2], 0.0)
                # bottom pad row of the shifted copy (read by DR #3 at k=6)
                nc.gpsimd.memset(
                    ts[:, SHIFT + (HP - 1) * RW : SHIFT + (HP - 1) * RW + W + 1],
                    0.0,
                )
                ts_bufs.append((ts, ts3))

            # warm the PE p-state while the first input DMAs stream: dummy
            # DoubleRow matmuls over the (already zeroed) warmup tile into a
            # psum bank that the first real group later resets (start=True)
            wu = cpool.tile([C, 1024], FP8)
            nc.gpsimd.memset(wu[:], 0.0)
            wu_ps = pspool.tile([C, 2 * NFLAT], F32, tag="ps1", name="ps1")
            wu_lhsT = wu[:, 0:256].rearrange("p (j m) -> p j m", j=2)
            for _ in range(6):
                nc.tensor.matmul(
                    wu_ps[:, 0:448], wu_lhsT,
                    wu[:, 0:896].rearrange("p (j n) -> p j n", j=2),
                    start=True, stop=True,
                    perf_mode=DR, skip_group_check=True,
                )

            def conv_group(ps, src, conv_idx, ks):
                """One fused group of 1-2 chunks in a 2-bank PSUM tile,
                interleaved tap-major (weight pairs consecutive per tap).

                Taps per chunk: DR c=0..2 pairs vertically adjacent taps
                (r0,c)+(r1,c) (planes at +RW). DR #3 pairs (r2,c0)+(r2,c1)
                using the col-shifted copy at +SHIFT. DR #4 pairs a
                zero-weight dummy plane with tap (r2,c2) (planes at +2*RW;
                dummy first so all reads stay in-bounds).
                """
                co = conv_idx * 1280
                pouts = [
                    ps[:, s * NFLAT : (s + 1) * NFLAT]
                    .rearrange("p (h w) -> p h w", w=RW)[:, :, 0:W]
                    for s in range(len(ks))
                ]
                # (weight col offset, rhs flat offset, plane stride)
                taps = [
                    (0, 0, RW),
                    (256, 1, RW),
                    (512, 2, RW),
                    (768, 2 * RW, SHIFT),
                    (1024, 2, 2 * RW),
                ]
                for t, (wc, ro, pstride) in enumerate(taps):
                    lhsT = w_sb[:, co + wc : co + wc + 256].rearrange(
                        "p (j m) -> p j m", j=2
                    )
                    for s, k in enumerate(ks):
                        rhs = bass.AP(
                            tensor=src.tensor,
                            offset=src.offset + k * CHUNK_ROWS * RW + ro,
                            ap=[src.ap[0], [pstride, 2], [RW, CHUNK_ROWS], [1, W]],
                        )
                        nc.tensor.matmul(
                            pouts[s], lhsT, rhs,
                            start=(t == 0), stop=(t == 4),
                            perf_mode=DR, skip_group_check=True,
                        )

            for i in range(BL):
                xs = xs0 if i == 0 else spool.tile([C, 2 * SHIFT], FP8, name="xs0")
                if i != 0:
                    nc.sync.dma_start(xs[:, 0:SHIFT], t1_d[i, :, 0:SHIFT])
                    nc.sync.dma_start(
                        xs[:, SHIFT : 2 * SHIFT], t1_d[i, :, 1 : SHIFT + 1]
                    )

                xb2 = rpool.tile([C, H, W], BF16, tag="xb2")
                nc.sync.dma_start(xb2[:], xb2_d[i])

                ts, ts3 = ts_bufs[i % 2]

                for ks in GROUPS:
                    h0 = ks[0] * CHUNK_ROWS
                    nr = len(ks) * CHUNK_ROWS
                    ps1 = pspool.tile([C, 2 * NFLAT], F32, tag="ps1", name="ps1")
                    conv_group(ps1, xs, 0, ks)
                    # bn1 + sign (hardtanh folded into sign) -> conv2 input
                    ps1v = ps1.rearrange(
                        "p (h w) -> p h w", w=RW
                    )[:, 0:nr, 0:W]
                    nc.scalar.activation(
                        ts3[:, 1 + h0 : 1 + h0 + nr, 1 : W + 1],
                        ps1v,
                        SIGN,
                        bias=bn_sb[:, 1:2],
                        scale=bn_sb[:, 0:1],
                    )
                    # shifted copy of the rows just produced
                    src = bass.AP(
                        tensor=ts.tensor,
                        offset=ts.offset + (1 + h0) * RW + 1,
                        ap=[ts.ap[0], [1, nr * RW]],
                    )
                    dst = bass.AP(
                        tensor=ts.tensor,
                        offset=ts.offset + SHIFT + (1 + h0) * RW,
                        ap=[ts.ap[0], [1, nr * RW]],
                    )
                    nc.sync.dma_start(dst, src)

                o = opool.tile([C, H, W], BF16)
                for ks in GROUPS:
                    h0 = ks[0] * CHUNK_ROWS
                    nr = len(ks) * CHUNK_ROWS
                    ps2 = pspool.tile([C, 2 * NFLAT], F32, tag="ps2", name="ps2")
                    conv_group(ps2, ts, 1, ks)
                    ps2v = ps2.rearrange(
                        "p (h w) -> p h w", w=RW
                    )[:, 0:nr, 0:W]
                    ov = o[:, h0 : h0 + nr, :]
                    # out = clip(ps2*inv2 + xb2, -1, 1); b2 pre-folded
                    nc.vector.affine_then_add(
                        ov, ps2v,
                        xb2[:, h0 : h0 + nr, :],
                        scale=bn_sb[:, 2:3], bias=0.0,
                    )
                    nc.vector.tensor_scalar(
                        ov, ov, 1.0, -1.0,
                        op0=mybir.AluOpType.min, op1=mybir.AluOpType.max,
                    )
                    if i == BL - 1:
                        # last image: per-group writes so the final DMA is
                        # small and starts right after the last clip
                        nc.sync.dma_start(y_d[i, :, h0 : h0 + nr, :], ov)
                if i != BL - 1:
                    nc.sync.dma_start(y_d[i], o[:])

    nc.compile()
    return nc


def _get_nc():
    global _NC_CACHE
    if _NC_CACHE is None:
        _NC_CACHE = _build_nc()
    return _NC_CACHE


def kernel(
    x, w1, w2, gamma1, beta1, mean1, var1, gamma2, beta2, mean2, var2,
    trace=False,
):
    x = np.ascontiguousarray(np.asarray(x, dtype=np.float32))
    w1 = np.asarray(w1, dtype=np.float32)
    w2 = np.asarray(w2, dtype=np.float32)

    # fold BN exactly as the reference does (f32 throughout)
    def fold(gamma, beta, mean, var):
        inv = (np.asarray(gamma, np.float32)
               / np.sqrt(np.asarray(var, np.float32) + np.float32(EPS)))
        b = np.asarray(beta, np.float32) - np.asarray(mean, np.float32) * inv
        return inv.astype(np.float32), b.astype(np.float32)

    inv1, b1 = fold(gamma1, beta1, mean1, var1)
    inv2, b2 = fold(gamma2, beta2, mean2, var2)
    bn_np = np.stack(
        [inv1, b1, inv2, np.zeros_like(inv2)], axis=1
    ).astype(np.float32)  # [C,4]

    # padded fp8 sign image of x: [B, C, HP*RW+16]; tail + pads zero
    t1_np = np.zeros((B, C, T1B), dtype=ml_dtypes.float8_e4m3fn)
    t1v = t1_np[:, :, :SHIFT].reshape(B, C, HP, RW)
    t1v[:, :, 1 : H + 1, 1 : W + 1] = np.sign(x)

    # bias-folded residual in bf16
    xb2_np = (x + b2[None, :, None, None]).astype(ml_dtypes.bfloat16)

    # fp8 weight tables; per conv 5 DoubleRow pair tables [cin, 2*cout]:
    #   c=0..2:  j=0 -> w[:,:,0,c].T, j=1 -> w[:,:,1,c].T
    #   pair 3:  j=0 -> w[:,:,2,0].T, j=1 -> w[:,:,2,1].T
    #   pair 4:  j=0 -> zeros (dummy), j=1 -> w[:,:,2,2].T
    w_np = np.zeros((C, WCOLS), dtype=ml_dtypes.float8_e4m3fn)
    for conv_idx, w in enumerate((w1, w2)):
        ws = np.sign(w).astype(ml_dtypes.float8_e4m3fn)  # [O, Cin, 3, 3]
        co = conv_idx * 1280
        for c in range(3):
            for j in range(2):
                w_np[:, co + c * 256 + j * 128 : co + c * 256 + (j + 1) * 128] = (
                    ws[:, :, j, c].T
                )
        w_np[:, co + 768 : co + 896] = ws[:, :, 2, 0].T
        w_np[:, co + 896 : co + 1024] = ws[:, :, 2, 1].T
        w_np[:, co + 1152 : co + 1280] = ws[:, :, 2, 2].T

    nc = _get_nc()
    in_maps = [
        {
            "t1": t1_np[i * BL : (i + 1) * BL],
            "xb2": xb2_np[i * BL : (i + 1) * BL],
            "w": w_np,
            "bn": bn_np,
        }
        for i in range(N_CORES)
    ]
    res = run_bass_kernel_spmd(
        nc, in_maps, core_ids=list(range(N_CORES)), trace=trace
    )
    y = np.concatenate(
        [np.asarray(res.results[i]["y"]) for i in range(N_CORES)], axis=0
    ).astype(np.float32)
    if trace:
        return y, res
    return y
